# revision 1
# baseline (speedup 1.0000x reference)
"""Blockwise transformer attention layer on 8 trn2 NeuronCores.

Math (per reference):
    q = (x @ Wq.T) / sqrt(D); k = x @ Wk.T; v = x @ Wv.T       (B,S,D), H=16 heads of Dh=64
    out = softmax(q k^T per head) @ v                           (no causal mask; scores ~ N(0,1/16)
                                                                 so exp without max-subtraction)
    y = out @ Wff.T + bff

Sharding: tensor-parallel over heads. 8 cores x 2 heads each. Each core:
  - computes qT,kT (transposed, [128=2*Dh, S]) and v (natural, [S,128]) for its 2 heads
    from the full xT and its weight slices,
  - attention with scores materialized TRANSPOSED ([k_pos, q_pos]) so exp(scores)
    feeds the o^T = v^T @ P accumulation directly (no PE transposes),
  - softmax denominator comes free from a ones-column appended to v,
  - partial final projection partial^T = Wff[:, slice].T-contraction, written transposed (bf16).
Host sums the 8 partials in fp32, transposes back, adds bias.

Schedule: emission interleaves the ACT(exp)-paced attention inner loops with the
QKV matmuls of the next batch / FF matmuls of the previous batch so the PE never
idles long enough for the HAM clock gate to re-throttle it.
"""

import numpy as np
import ml_dtypes

BF16 = ml_dtypes.bfloat16

B, S, D = 2, 2048, 1024
DH = 64          # head dim
HPC = 2          # heads per core
NCORES = 8
NQ = 512         # q-chunk width (psum bank width in fp32)


def _interleave(primary, filler, skip=0):
    """Emit primary units in order with filler units spread evenly between
    them, starting after the first `skip` primary units."""
    out = []
    np_, nf = len(primary), len(filler)
    span = max(np_ - skip, 1)
    fi = 0
    for i, u in enumerate(primary):
        out.append(u)
        want = max(i + 1 - skip, 0) * nf // span
        while fi < want:
            out.append(filler[fi])
            fi += 1
    out.extend(filler[fi:])
    return out


def build_program(b=B, s=S, d=D, num_devices=NCORES, debug=False):
    import concourse.bass as bass
    import concourse.tile as tile
    from concourse import bacc, mybir
    from concourse._compat import get_trn_type
    from contextlib import ExitStack

    f32 = mybir.dt.float32
    bf16 = mybir.dt.bfloat16
    Exp = mybir.ActivationFunctionType.Exp

    KC = d // 128           # contraction chunks over D
    SQ = s // NQ            # q chunks
    SB = s // 128           # k blocks
    VW = DH + 1             # v block cols per head (64 dims + ones col)
    NG = HPC * SQ           # normalization groups per batch

    nc = bacc.Bacc(
        get_trn_type() or "TRN2",
        target_bir_lowering=False,
        debug=debug,
        num_devices=num_devices,
    )

    xT = nc.dram_tensor("xT", (b, KC, 128, s), bf16, kind="ExternalInput")
    wqT = nc.dram_tensor("wqT", (KC, 128, 128), bf16, kind="ExternalInput")
    wkT = nc.dram_tensor("wkT", (KC, 128, 128), bf16, kind="ExternalInput")
    wvT = nc.dram_tensor("wvT", (KC, 128, 128), bf16, kind="ExternalInput")
    wfT = nc.dram_tensor("wfT", (128, KC, 128), bf16, kind="ExternalInput")
    outp = nc.dram_tensor("outp", (b, KC, 128, s), bf16, kind="ExternalOutput")

    with tile.TileContext(nc) as tc, ExitStack() as ctx:
        const = ctx.enter_context(tc.tile_pool(name="const", bufs=1))
        xpool = ctx.enter_context(tc.tile_pool(name="xp", bufs=2))
        proj = ctx.enter_context(tc.tile_pool(name="proj", bufs=2))
        work = ctx.enter_context(tc.tile_pool(name="work", bufs=3))
        osbp = ctx.enter_context(tc.tile_pool(name="osb", bufs=6))
        opool = ctx.enter_context(tc.tile_pool(name="op", bufs=3))
        psum = ctx.enter_context(
            tc.tile_pool(name="ps", bufs=1, space=bass.MemorySpace.PSUM)
        )

        wq_sb = const.tile([128, KC, 128], bf16, tag="wq")
        wk_sb = const.tile([128, KC, 128], bf16, tag="wk")
        wv_sb = const.tile([128, KC, 128], bf16, tag="wv")
        wf_sb = const.tile([128, KC, 128], bf16, tag="wf")
        nc.sync.dma_start(out=wq_sb, in_=wqT[:].rearrange("k p m -> p k m"))
        nc.sync.dma_start(out=wk_sb, in_=wkT[:].rearrange("k p m -> p k m"))
        nc.sync.dma_start(out=wv_sb, in_=wvT[:].rearrange("k p m -> p k m"))
        nc.sync.dma_start(out=wf_sb, in_=wfT[:])

        st = [dict() for _ in range(b)]

        NXH = 2 if s >= 2 * NQ else 1   # x s-split factor
        HS = s // NXH

        # distinct DGE queues per (batch, s-half) so descriptor setup for the
        # 32 input-chunk DMAs runs in parallel instead of serializing on one
        # sequencer (~0.6us each)
        _XQ = {}

        def load_x(ib):
            # split by s-half so the first QKV chunks start after ~1/2 the load
            chunks = {}
            hs = HS
            for sh in range(NXH):
                eng = getattr(nc, _XQ.get((ib % 2, sh % 2), "sync"))
                for kc in range(KC):
                    xc = xpool.tile(
                        [128, hs], bf16, tag=f"x{kc}_{sh}", name="x_chunk"
                    )
                    eng.dma_start(out=xc, in_=xT[ib, kc, :, sh * hs : (sh + 1) * hs])
                    chunks[(kc, sh)] = xc
            st[ib]["x"] = chunks

        def alloc_qkv(ib):
            st[ib]["qT"] = proj.tile([128, s], bf16, tag="qT", name="qT")
            st[ib]["kT"] = proj.tile([128, s], bf16, tag="kT", name="kT")
            st[ib]["v"] = proj.tile([128, SB, HPC * VW], bf16, tag="v", name="v_sb")

        # ---- QKV projection units ------------------------------------------
        def qk_chunk(ib, which, sc):
            w_sb = wq_sb if which == "q" else wk_sb

            def emit():
                x_sb = st[ib]["x"]
                dst = st[ib][which + "T"]
                sh, off = divmod(sc * NQ, HS)
                ps = psum.tile([128, NQ], f32, tag="mm", bufs=2, name="mm_ps")
                for kc in range(KC):
                    nc.tensor.matmul(
                        ps, w_sb[:, kc, :], x_sb[(kc, sh)][:, off : off + NQ],
                        start=(kc == 0), stop=(kc == KC - 1),
                    )
                nc.vector.tensor_copy(out=dst[:, sc * NQ : (sc + 1) * NQ], in_=ps)
            return emit

        def v_units(ib):
            units = []

            def ones_cols():
                v_sb = st[ib]["v"]
                nc.vector.memset(v_sb[:, :, DH : DH + 1], 1.0)
                nc.vector.memset(v_sb[:, :, DH + VW : DH + VW + 1], 1.0)

            units.append(ones_cols)

            def v_block(sbi):
                def emit():
                    x_sb = st[ib]["x"]
                    v_sb = st[ib]["v"]
                    sh, off = divmod(sbi * 128, HS)
                    ps = psum.tile([128, 128], f32, tag="mm", bufs=2, name="mm_ps")
                    for kc in range(KC):
                        nc.tensor.matmul(
                            ps, x_sb[(kc, sh)][:, off : off + 128], wv_sb[:, kc, :],
                            start=(kc == 0), stop=(kc == KC - 1),
                        )
                    nc.vector.tensor_copy(
                        out=v_sb[:, sbi, 0:DH], in_=ps[:, 0:DH]
                    )
                    nc.vector.tensor_copy(
                        out=v_sb[:, sbi, VW : VW + DH], in_=ps[:, DH : 2 * DH]
                    )
                return emit

            for sbi in range(SB):
                units.append(v_block(sbi))
            return units

        # ---- attention group (h, qc): 8 pair-steps + finalize --------------
        def group_units(ib, h, qc):
            hsl = slice(h * DH, (h + 1) * DH)
            qsl = slice(qc * NQ, (qc + 1) * NQ)
            g = {}
            units = []

            def step(kp):
                # 2 score matmuls + one [128, 2*NQ] exp; attnV of the
                # previous pair is emitted after the scores so the PE
                # never waits on the current exp
                def emit():
                    qT, kT, v_sb = st[ib]["qT"], st[ib]["kT"], st[ib]["v"]
                    if kp == 0:
                        g["o"] = psum.tile(
                            [DH + 1, NQ], f32, tag="o", bufs=2, name="o_ps"
                        )
                        g["prev"] = None
                    s2 = psum.tile([128, 2 * NQ], f32, tag="s", bufs=2, name="s2_ps")
                    for half in range(2):
                        kb = 2 * kp + half
                        nc.tensor.matmul(
                            s2[:, half * NQ : (half + 1) * NQ],
                            kT[hsl, kb * 128 : (kb + 1) * 128],
                            qT[hsl, qsl],
                            start=True, stop=True,
                        )
                    p2 = work.tile([128, 2 * NQ], bf16, tag="p", bufs=4, name="p2")
                    nc.scalar.activation(out=p2, in_=s2, func=Exp)
                    if g["prev"] is not None:
                        pkp, pp = g["prev"]
                        for half in range(2):
                            kb = 2 * pkp + half
                            nc.tensor.matmul(
                                g["o"], v_sb[:, kb, h * VW : (h + 1) * VW],
                                pp[:, half * NQ : (half + 1) * NQ],
                                start=(kb == 0), stop=False,
                            )
                    g["prev"] = (kp, p2)
                return emit

            def fin():
                def emit():
                    v_sb = st[ib]["v"]
                    pkp, pp = g["prev"]
                    for half in range(2):
                        kb = 2 * pkp + half
                        nc.tensor.matmul(
                            g["o"], v_sb[:, kb, h * VW : (h + 1) * VW],
                            pp[:, half * NQ : (half + 1) * NQ],
                            start=False, stop=(half == 1),
                        )
                    o_sb = osbp.tile([DH + 1, NQ], f32, tag="osb", name="o_sb")
                    nc.vector.tensor_copy(out=o_sb, in_=g["o"])
                    st[ib][("o", h, qc)] = o_sb
                return emit

            for kp in range(SB // 2):
                units.append(step(kp))
            units.append(fin())
            return units

        # ---- per-qc normalization + final projection -----------------------
        def norm_qc(ib, qc):
            def emit():
                qsl = slice(qc * NQ, (qc + 1) * NQ)
                ffr = st[ib]["ffr"]
                for h in range(HPC):
                    o_sb = st[ib][("o", h, qc)]
                    dnrow = work.tile([1, NQ], f32, tag="dnrow", bufs=3, name="dnrow")
                    nc.gpsimd.dma_start(out=dnrow, in_=o_sb[DH : DH + 1, :])
                    rr = work.tile([1, NQ], f32, tag="rr", bufs=3, name="rr")
                    nc.vector.reciprocal_approx_fast(out=rr, in_=dnrow)
                    rdbc = work.tile([DH, NQ], f32, tag="rdbc", bufs=3, name="rdbc")
                    nc.gpsimd.partition_broadcast(rdbc, rr)
                    nc.vector.tensor_mul(
                        out=ffr[h * DH : (h + 1) * DH, qsl],
                        in0=o_sb[0:DH, :],
                        in1=rdbc,
                    )
            return emit

        def ff_qc(ib, qc):
            units = []

            def one(j):
                def emit():
                    qsl = slice(qc * NQ, (qc + 1) * NQ)
                    ps = psum.tile([128, NQ], f32, tag="mm", bufs=2, name="mm_ps")
                    nc.tensor.matmul(
                        ps, wf_sb[:, j, :], st[ib]["ffr"][:, qsl],
                        start=True, stop=True,
                    )
                    f_sb = opool.tile([128, NQ], bf16, tag="f", name="f_sb")
                    nc.vector.tensor_copy(out=f_sb, in_=ps)
                    nc.sync.dma_start(out=outp[ib, j, :, qsl], in_=f_sb)
                return emit

            for j in range(KC):
                units.append(one(j))
            return units

        def warm_unit(ib):
            # write-only matmul that keeps the PE's HAM activity window busy
            # through ACT-paced stretches with no real filler work; without it
            # the clock gate drops the PE to 1.2 GHz and the whole stretch
            # becomes PE-bound at half clock
            def emit():
                zp = psum.tile([128, NQ], f32, tag="mm", bufs=2, name="warm")
                nc.tensor.matmul(
                    zp, wq_sb[:, 0, :], st[ib]["qT"][:, 0:NQ],
                    start=True, stop=True,
                )
            return emit

        # ---- attention stream: qc-major with streamed norm/ff --------------
        def attn_stream(ib, deferred_qT=False, warm_qcs=(), qc0_extra=None):
            st[ib]["ffr"] = proj.tile([128, s], bf16, tag="ffr", name="ffr")
            qc_blocks = []
            for qc in range(SQ):
                g0 = group_units(ib, 0, qc)
                g1 = group_units(ib, 1, qc)
                if qc == 0 and qc0_extra:
                    # splice remaining qkv units after the step that unblocks
                    # them, so the first group starts as soon as the first
                    # k-chunk/v-blocks exist instead of after the whole
                    # projection prologue
                    merged = []
                    for kp in range(SB // 2):
                        merged.append(g0[kp])
                        merged.extend(qc0_extra[kp])
                    g0 = merged + [g0[-1]]
                # h1's first score-pair ahead of h0's finalize keeps the exp
                # stream dense across the group boundary
                attn = g0[:-1] + [g1[0], g0[-1]] + g1[1:]
                if deferred_qT and qc + 1 < SQ:
                    attn.append(qk_chunk(ib, "q", qc + 1))
                if qc in warm_qcs:
                    attn = _interleave(attn, [warm_unit(ib) for _ in range(10)], skip=1)
                tail = [norm_qc(ib, qc)] + ff_qc(ib, qc)
                qc_blocks.append((attn, tail))
            # spread each qc's norm/ff tail thinly across the next qc's steps
            units = []
            pend = None
            for attn, tail in qc_blocks:
                if pend:
                    units.extend(_interleave(attn, pend, skip=2))
                else:
                    units.extend(attn)
                pend = tail
            units.extend(pend)
            return units

        # ---- emission schedule ---------------------------------------------
        load_x(0)
        alloc_qkv(0)
        vu0 = v_units(0)
        ones0, vb0 = vu0[0], vu0[1:]
        # minimal prologue: everything group (h0, qc=0) consumes, in need-order
        prologue = [ones0, qk_chunk(0, "k", 0)]
        prologue.extend(vb0[0:4])
        prologue.append(qk_chunk(0, "q", 0))
        for i in range(1, SQ):
            prologue.append(qk_chunk(0, "k", i))
            prologue.extend(vb0[4 * i : 4 * (i + 1)])
        for u in prologue:
            u()
        qc0_extra = None

        if b > 1:
            load_x(1)
            alloc_qkv(1)
            vu = v_units(1)
            ones_u, vb = vu[0], vu[1:]
            # remaining b0 q-chunks first (needed from qc=1 on), then batch-1
            # qkv in the order its attention will need it
            fillers = [qk_chunk(0, "q", 1), qk_chunk(0, "q", 2), qk_chunk(0, "q", 3)]
            fillers += [ones_u, qk_chunk(1, "k", 0)]
            fillers.extend(vb[0:4])
            fillers.append(qk_chunk(1, "q", 0))
            for i in range(1, SQ):
                fillers.append(qk_chunk(1, "k", i))
                fillers.extend(vb[4 * i : 4 * (i + 1)])
            s0 = attn_stream(0, qc0_extra=qc0_extra)
            head0, tail0 = s0[:-9], s0[-9:]
            for u in _interleave(head0, fillers):
                u()
            s1 = attn_stream(1, deferred_qT=True, warm_qcs=(1, 2, 3))
            for u in _interleave(s1[:27], tail0):
                u()
            for u in s1[27:]:
                u()
        else:
            for u in attn_stream(0, qc0_extra=qc0_extra):
                u()

    nc.compile()
    return nc


def make_in_maps(x, Wq, Wk, Wv, Wff, n_cores=NCORES):
    """Per-core input dicts. Core c owns heads (2c, 2c+1) = D dims [128c, 128c+128)."""
    x = np.asarray(x, dtype=np.float32)
    b, s, d = x.shape
    KC = d // 128
    xT = np.ascontiguousarray(x.transpose(0, 2, 1)).reshape(b, KC, 128, s).astype(BF16)
    scale = 1.0 / np.sqrt(d)
    in_maps = []
    for c in range(n_cores):
        sl = slice(128 * c, 128 * (c + 1))
        wq = np.ascontiguousarray((np.asarray(Wq)[sl, :] * scale).T).reshape(KC, 128, 128)
        wk = np.ascontiguousarray(np.asarray(Wk)[sl, :].T).reshape(KC, 128, 128)
        wv = np.ascontiguousarray(np.asarray(Wv)[sl, :].T).reshape(KC, 128, 128)
        wf = np.ascontiguousarray(np.asarray(Wff)[:, sl].T).reshape(128, KC, 128)
        in_maps.append(
            {
                "xT": xT,
                "wqT": wq.astype(BF16),
                "wkT": wk.astype(BF16),
                "wvT": wv.astype(BF16),
                "wfT": wf.astype(BF16),
            }
        )
    return in_maps


def gather(results, bff, b=B, s=S, d=D):
    total = np.zeros((b, d // 128, 128, s), np.float32)
    for r in results:
        total += r["outp"].astype(np.float32)
    out = total.reshape(b, d, s).transpose(0, 2, 1)
    return (out + np.asarray(bff, np.float32)[None, None, :]).astype(np.float32)


_CACHE = {}


def kernel(x, Wq, Wk, Wv, Wff, bff):
    from concourse.bass_utils import run_bass_kernel_spmd

    x = np.asarray(x, np.float32)
    b, s, d = x.shape
    key = (b, s, d)
    if key not in _CACHE:
        _CACHE[key] = build_program(b, s, d)
    nc = _CACHE[key]
    in_maps = make_in_maps(x, Wq, Wk, Wv, Wff)
    res = run_bass_kernel_spmd(nc, in_maps, list(range(NCORES)))
    return gather(res.results, bff, b, s, d)



# revision 49
# speedup vs baseline: 1.3610x; 1.3610x over previous
"""Blockwise transformer attention layer on 8 trn2 NeuronCores.

Math (per reference):
    q = (x @ Wq.T) / sqrt(D); k = x @ Wk.T; v = x @ Wv.T       (B,S,D), H=16 heads of Dh=64
    out = softmax(q k^T per head) @ v                           (no causal mask; scores ~ N(0,1/16)
                                                                 so exp without max-subtraction)
    y = out @ Wff.T + bff

Sharding: tensor-parallel over heads. 8 cores x 2 heads each. Each core:
  - computes qT,kT (transposed, [128=2*Dh, S]) and v (natural, [S,130]) for its 2 heads
    from the full xT and its weight slices,
  - attention with scores materialized TRANSPOSED ([k_pos, q_pos]) so exp(scores)
    feeds the o^T = v^T @ P accumulation directly (no PE transposes),
  - softmax denominator comes free from a ones-column appended to v,
  - partial final projection partial^T = Wff[:, slice].T-contraction, written transposed (bf16).
Host sums the 8 partials in fp32, transposes back, adds bias.

Key trn2 scheduling facts this version exploits:
  - The 128x128 PE array executes matmuls with disjoint row-group footprints
    CONCURRENTLY (tile_position row tiling). The two heads' score matmuls
    (contraction = head dim 64) are emitted back-to-back with h0 on PE rows
    0-63 and h1 on rows 64-127, writing adjacent psum banks: measured 2.0x
    throughput vs sequential on HW. This halves score-matmul time and makes
    the kernel ACT(exp)-bound.
  - ACT exp costs ~(N+352)/1.2 ns + ~185 ns access overhead per instruction,
    so exp is issued as [128, 1024] (both heads of one k-block) instructions.
  - The attention stream is paced by ACT; QKV/FF matmuls of the other batch
    are interleaved as single-matmul filler units between steps so the PE
    fills its idle time without ever delaying the next score pair by more
    than ~one matmul.
"""

import numpy as np
import ml_dtypes

BF16 = ml_dtypes.bfloat16

B, S, D = 2, 2048, 1024
DH = 64          # head dim
HPC = 2          # heads per core
NCORES = 8
NQ = 512         # q-chunk width (psum bank width in fp32)


def _interleave(primary, filler, skip=0):
    """Emit primary units in order with filler units spread evenly between
    them, starting after the first `skip` primary units."""
    out = []
    np_, nf = len(primary), len(filler)
    span = max(np_ - skip, 1)
    fi = 0
    for i, u in enumerate(primary):
        out.append(u)
        want = max(i + 1 - skip, 0) * nf // span
        while fi < want:
            out.append(filler[fi])
            fi += 1
    out.extend(filler[fi:])
    return out


def build_program(b=B, s=S, d=D, num_devices=NCORES, debug=False):
    import concourse.bass as bass
    import concourse.tile as tile
    from concourse import bacc, mybir
    from concourse._compat import get_trn_type
    from contextlib import ExitStack

    f32 = mybir.dt.float32
    bf16 = mybir.dt.bfloat16
    Exp = mybir.ActivationFunctionType.Exp

    KC = d // 128           # contraction chunks over D
    SQ = s // NQ            # q chunks
    SB = s // 128           # k blocks
    VW = DH + 1             # v block cols per head (64 dims + ones col)

    nc = bacc.Bacc(
        get_trn_type() or "TRN2",
        target_bir_lowering=False,
        debug=debug,
        num_devices=num_devices,
    )

    xT = nc.dram_tensor("xT", (b, KC, 128, s), bf16, kind="ExternalInput")
    wqT = nc.dram_tensor("wqT", (128, KC, 128), bf16, kind="ExternalInput")
    wkT = nc.dram_tensor("wkT", (128, KC, 128), bf16, kind="ExternalInput")
    wvT = nc.dram_tensor("wvT", (128, KC, 128), bf16, kind="ExternalInput")
    wfT = nc.dram_tensor("wfT", (128, KC, 128), bf16, kind="ExternalInput")
    outp = nc.dram_tensor("outp", (b, KC, 128, s), bf16, kind="ExternalOutput")

    with tile.TileContext(nc) as tc, ExitStack() as ctx:
        sb = ctx.enter_context(tc.tile_pool(name="sb", bufs=1))
        const = xpool = proj = work = osbp = opool = sb
        psum = ctx.enter_context(
            tc.tile_pool(name="ps", bufs=1, space=bass.MemorySpace.PSUM)
        )

        # weights are host-laid as (128, KC, 128) contiguous so each DMA is
        # one 2KB descriptor per partition; wk/wq first (k0/q0 + PE warmup
        # gate on them), wv/wf behind the x quarters they're needed after
        wq_sb = const.tile([128, KC, 128], bf16, tag="wq")
        wk_sb = const.tile([128, KC, 128], bf16, tag="wk")
        wv_sb = const.tile([128, KC, 128], bf16, tag="wv")
        wf_sb = const.tile([128, KC, 128], bf16, tag="wf")
        nc.scalar.dma_start(out=wk_sb, in_=wkT[:])
        nc.scalar.dma_start(out=wq_sb, in_=wqT[:])
        nc.gpsimd.dma_start(out=wv_sb, in_=wvT[:])

        st = [dict() for _ in range(b)]

        NXH = 2 if s >= 2 * NQ else 1   # x s-split factor
        HS = s // NXH

        def x_qtr_unit(ib, qt, eng):
            # one s-quarter of x for all KC chunks as a single ~1MB DMA —
            # big DMAs fan out over all 16 SDMA engines (~350 GB/s) instead
            # of paying ~2us completion latency per small chunk
            def emit():
                xc = xpool.tile([128, KC, NQ], bf16, tag=f"x{qt}", bufs=2, name="x_qtr")
                eng.dma_start(
                    out=xc,
                    in_=xT[ib, :, :, qt * NQ : (qt + 1) * NQ].rearrange(
                        "k p s -> p k s"
                    ),
                )
                st[ib]["x"][qt] = xc
            return emit

        def sync_gate():
            # tiny dummy DMA on the sync queue whose input depends on
            # batch-0's kT chunk 1 — holds the queue so batch-1's x
            # transfers can't race ahead and steal ramp HBM bandwidth
            def emit():
                g = work.tile([1, 64], bf16, tag="gate", bufs=1, name="gate")
                nc.sync.dma_start(out=g, in_=st[0]["kT"][0:1, NQ : NQ + 64])
            return emit

        def alloc_qkv(ib):
            st[ib]["qT"] = proj.tile([128, s], bf16, tag="qT", bufs=2, name="qT")
            st[ib]["kT"] = proj.tile([128, s], bf16, tag="kT", bufs=2, name="kT")
            st[ib]["v"] = proj.tile([128, SB, HPC * VW], bf16, tag="v", bufs=2, name="v_sb")
            st[ib]["ffr"] = proj.tile([128, s], bf16, tag="ffr", bufs=2, name="ffr")

        # ---- QKV projection units (single-matmul granularity) --------------
        def qk_mm(ib, which, sc, kc):
            w_sb = wq_sb if which == "q" else wk_sb

            def emit():
                x_sb = st[ib]["x"]
                if kc == 0:
                    st[ib][("mm", which, sc)] = psum.tile(
                        [128, NQ], f32, tag="mm", bufs=2, name="mm_ps"
                    )
                nc.tensor.matmul(
                    st[ib][("mm", which, sc)],
                    w_sb[:, kc, :], x_sb[sc][:, kc, :],
                    start=(kc == 0), stop=(kc == KC - 1),
                )
            return emit

        def qk_cast(ib, which, sc):
            def emit():
                dst = st[ib][which + "T"]
                nc.vector.tensor_copy(
                    out=dst[:, sc * NQ : (sc + 1) * NQ],
                    in_=st[ib].pop(("mm", which, sc)),
                )
            return emit

        def qk_units(ib, which, sc):
            return [qk_mm(ib, which, sc, kc) for kc in range(KC)] + [qk_cast(ib, which, sc)]

        def ones_unit(ib):
            def emit():
                v_sb = st[ib]["v"]
                nc.vector.memset(v_sb[:, :, DH : DH + 1], 1.0)
                nc.vector.memset(v_sb[:, :, DH + VW : DH + VW + 1], 1.0)
            return emit

        def v_mm(ib, sbi, kc):
            def emit():
                x_sb = st[ib]["x"]
                qt, off = divmod(sbi * 128, NQ)
                if kc == 0:
                    st[ib][("mm", "v", sbi)] = psum.tile(
                        [128, 128], f32, tag="mm", bufs=2, name="mm_ps"
                    )
                nc.tensor.matmul(
                    st[ib][("mm", "v", sbi)],
                    x_sb[qt][:, kc, off : off + 128], wv_sb[:, kc, :],
                    start=(kc == 0), stop=(kc == KC - 1),
                )
            return emit

        def v_cast(ib, sbi):
            def emit():
                v_sb = st[ib]["v"]
                ps = st[ib].pop(("mm", "v", sbi))
                # both heads in one strided copy: [128,(2,64)] -> cols
                # (0:64, 65:129) of the (2, VW)-strided v row
                nc.vector.tensor_copy(
                    out=v_sb[:, sbi, 0 : 2 * VW].rearrange(
                        "p (h w) -> p h w", h=2
                    )[:, :, 0:DH],
                    in_=ps.rearrange("p (h w) -> p h w", h=2),
                )
            return emit

        def v_units(ib, sbi):
            return [v_mm(ib, sbi, kc) for kc in range(KC)] + [v_cast(ib, sbi)]

        # ---- attention steps: both heads per k-block -----------------------
        def attn_step(ib, qc, kb):
            # concurrent score pair (h0 rows 0-63 -> bank 0, h1 rows 64-127
            # -> bank 1 of one 2-bank s2 tile), one [128, 1024] exp, and the
            # previous k-block's attnV pair behind the scores so the PE never
            # waits on the current exp.
            qsl = slice(qc * NQ, (qc + 1) * NQ)

            def emit():
                g = st[ib]["g"]
                qT, kT, v_sb = st[ib]["qT"], st[ib]["kT"], st[ib]["v"]
                if kb == 0:
                    g["o0"] = psum.tile([VW, NQ], f32, tag="o0", bufs=1, name="o0_ps")
                    g["o1"] = psum.tile([VW, NQ], f32, tag="o1", bufs=1, name="o1_ps")
                    g["prev"] = None
                s2 = psum.tile([128, 2 * NQ], f32, tag="s", bufs=2, name="s2_ps")
                ksl = slice(kb * 128, (kb + 1) * 128)
                nc.tensor.matmul(
                    s2[:, 0:NQ], kT[0:DH, ksl], qT[0:DH, qsl],
                    start=True, stop=True, tile_position=(0, 0),
                )
                nc.tensor.matmul(
                    s2[:, NQ : 2 * NQ], kT[DH:128, ksl], qT[DH:128, qsl],
                    start=True, stop=True, tile_position=(64, 0),
                )
                p2 = work.tile([128, 2 * NQ], bf16, tag="p", bufs=4, name="p2")
                nc.scalar.activation(out=p2, in_=s2, func=Exp)
                if g["prev"] is not None:
                    pkb, pp = g["prev"]
                    for h in range(2):
                        nc.tensor.matmul(
                            g[f"o{h}"], v_sb[:, pkb, h * VW : (h + 1) * VW],
                            pp[:, h * NQ : (h + 1) * NQ],
                            start=(pkb == 0), stop=False,
                        )
                g["prev"] = (kb, p2)
            return emit

        def attn_fin(ib, qc, par=False):
            def emit():
                g = st[ib]["g"]
                v_sb = st[ib]["v"]
                pkb, pp = g["prev"]
                for h in range(2):
                    nc.tensor.matmul(
                        g[f"o{h}"], v_sb[:, pkb, h * VW : (h + 1) * VW],
                        pp[:, h * NQ : (h + 1) * NQ],
                        start=(pkb == 0), stop=True,
                    )
                for h in range(2):
                    o_sb = osbp.tile([VW, NQ], f32, tag=f"osb{h}", bufs=2, name="o_sb")
                    if par and h == 1:
                        nc.scalar.copy(out=o_sb, in_=g[f"o{h}"])
                    else:
                        nc.vector.tensor_copy(out=o_sb, in_=g[f"o{h}"])
                    st[ib][("o", h, qc)] = o_sb
                g["prev"] = None
            return emit

        # ---- per-qc normalization + final projection -----------------------
        def norm_unit(ib, qc, h, dn_eng=None, mul_eng=None):
            def emit():
                qsl = slice(qc * NQ, (qc + 1) * NQ)
                ffr = st[ib]["ffr"]
                o_sb = st[ib].pop(("o", h, qc))
                dnrow = work.tile([1, NQ], f32, tag="dnrow", bufs=3, name="dnrow")
                (dn_eng or nc.gpsimd).dma_start(out=dnrow, in_=o_sb[DH : DH + 1, :])
                rr = work.tile([1, NQ], f32, tag="rr", bufs=3, name="rr")
                nc.vector.reciprocal_approx_fast(out=rr, in_=dnrow)
                rdbc = work.tile([DH, NQ], f32, tag="rdbc", bufs=3, name="rdbc")
                nc.gpsimd.partition_broadcast(rdbc, rr)
                (mul_eng or nc.vector).tensor_mul(
                    out=ffr[h * DH : (h + 1) * DH, qsl],
                    in0=o_sb[0:DH, :],
                    in1=rdbc,
                )
            return emit

        def ff_unit(ib, qc, j, cast_eng=None):
            def emit():
                qsl = slice(qc * NQ, (qc + 1) * NQ)
                ps = psum.tile([128, NQ], f32, tag="mm", bufs=2, name="mm_ps")
                nc.tensor.matmul(
                    ps, wf_sb[:, j, :], st[ib]["ffr"][:, qsl],
                    start=True, stop=True,
                )
                f_sb = opool.tile([128, NQ], bf16, tag="f", bufs=3, name="f_sb")
                if cast_eng is None:
                    nc.vector.tensor_copy(out=f_sb, in_=ps)
                else:
                    cast_eng.copy(out=f_sb, in_=ps)
                nc.sync.dma_start(out=outp[ib, j, :, qsl], in_=f_sb)
            return emit

        def warm_unit(n=4):
            # keeps the PE's HAM activity window busy through stretches with
            # no real matmul work (ramp, final tail) so it doesn't drop to
            # 1.2 GHz; writes a scratch psum bank from weight data
            def emit():
                ws = psum.tile([128, 2 * NQ], f32, tag="s", bufs=2, name="warm")
                for _ in range(n):
                    nc.tensor.matmul(
                        ws[:, 0:NQ], wk_sb[:, 0, :],
                        wk_sb[:, 0:4, :], start=True, stop=True,
                    )
            return emit

        # ---- stream assembly ----------------------------------------------
        def attn_qc_units(ib, qc):
            units = [attn_step(ib, qc, kb) for kb in range(SB)]
            units.append(attn_fin(ib, qc))
            return units

        def weave(steps, slots):
            """steps: 17 primary units; slots: dict step_idx -> [units] to
            emit right after that step (idx 'end' after everything; numeric
            slots past the last step flush at the end, in order)."""
            out = []
            for i, s_ in enumerate(steps):
                out.append(s_)
                out.extend(slots.get(i, ()))
            for i in sorted(k for k in slots if k != "end" and k >= len(steps)):
                out.extend(slots[i])
            out.extend(slots.get("end", ()))
            return out

        def add_tail_slots(slots, ib, qc, ff_start=10):
            # previous qc's tail woven into this qc: norms immediately (their
            # chain is ~4us deep), ffs only after the chain has drained so a
            # queued ff matmul never blocks the PE queue
            slots.setdefault(0, []).append(norm_unit(ib, qc, 0))
            slots.setdefault(1, []).append(norm_unit(ib, qc, 1))
            for j in range(KC):
                slots.setdefault(ff_start + j, []).append(ff_unit(ib, qc, j))

        def spread(slots, units, lo, hi):
            # distribute units into slots lo..hi in consecutive chunks
            # (preserves relative order)
            nsl = hi - lo + 1
            per = (len(units) + nsl - 1) // nsl
            for i in range(nsl):
                chunk = units[i * per : (i + 1) * per]
                if chunk:
                    slots.setdefault(lo + i, []).extend(chunk)

        # batch 0 prologue: x quarter 0 (the only x the first score needs)
        # races on scalar next to wk/wq; quarters 1-3 + wv/wf follow on
        # sync/gpsimd. ACT table preload + PE warm matmuls run during the
        # DMA wait; k0/q0 matmuls interleave so the exp stream starts
        # ~1.5us after quarter 0 lands.
        st[0]["x"] = {}
        alloc_qkv(0)
        st[0]["g"] = {}
        # all batch-0 x quarters serialized on the scalar HWDGE ring in
        # need-order so quarter 0 gets the full HBM bandwidth first; wk
        # alone on sync, wv/wf on gpsimd SWDGE
        def act_preload():
            # preload the exp table + the gpsimd Q7 library (its one-time
            # LOAD_LIB otherwise stalls the first norm chain ~5us)
            scratch = work.tile([1, 8], bf16, tag="actw", bufs=1, name="actw")
            nc.scalar.activation(out=scratch, in_=wk_sb[0:1, 0, 0:8], func=Exp)
            bscr = work.tile([2, 8], f32, tag="bscr", bufs=1, name="bscr")
            nc.vector.memset(scratch2 := work.tile([1, 8], f32, tag="bsrc", bufs=1, name="bsrc"), 1.0)
            nc.gpsimd.partition_broadcast(bscr, scratch2)

        act_preload()
        warm_unit(14)()
        x_qtr_unit(0, 0, nc.scalar)()
        for qt in range(1, SQ):
            x_qtr_unit(0, qt, nc.sync)()
        nc.gpsimd.dma_start(out=wf_sb, in_=wfT[:])
        k0u, q0u = qk_units(0, "k", 0), qk_units(0, "q", 0)
        pro = [ones_unit(0)]
        for i in range(KC):
            pro += [k0u[i], q0u[i]]
        pro += [k0u[KC], q0u[KC]]
        for u in pro:
            u()

        # qc0 hand-laid: v block kb right after step kb (attnV(kb) runs at
        # step kb+1), k chunk c spread over the 3 steps before step 4c,
        # q1 late.
        slots = {}
        for kb in range(16):
            slots.setdefault(kb, []).extend(v_units(0, kb))
        for pos, c in ((0, 1), (4, 2), (8, 3)):
            ku = qk_units(0, "k", c)
            for i in range(3):
                slots[pos + i].extend(ku[3 * i : 3 * (i + 1)])
        q1u = qk_units(0, "q", 1)
        slots[12].extend(q1u[0:5])
        slots[13].extend(q1u[5:])
        for u in weave(attn_qc_units(0, 0), slots):
            u()

        # qc1: qc0 tails + q2 + batch-1 x quarter 0 + k0/q0
        slots = {}
        add_tail_slots(slots, 0, 0)
        extra = qk_units(0, "q", 2)
        if b > 1:
            alloc_qkv(1)
            st[1]["x"] = {}
            st[1]["g"] = {}
            extra += [ones_unit(1), sync_gate(), x_qtr_unit(1, 0, nc.sync)]
            k0b, q0b = qk_units(1, "k", 0), qk_units(1, "q", 0)
            for kc in range(KC):
                extra += [k0b[kc], q0b[kc]]
            extra += [k0b[KC], q0b[KC]]
        spread(slots, extra, 2, 15)
        for u in weave(attn_qc_units(0, 1), slots):
            u()

        # qc2: qc1 tails + q3 + batch-1 x quarter 1, v0-v3, k1
        slots = {}
        add_tail_slots(slots, 0, 1)
        extra = qk_units(0, "q", 3)
        if b > 1:
            extra += [x_qtr_unit(1, 1, nc.sync)]
            extra += v_units(1, 0) + v_units(1, 1)
            extra += qk_units(1, "k", 1)
            extra += v_units(1, 2) + v_units(1, 3)
        spread(slots, extra, 2, 15)
        for u in weave(attn_qc_units(0, 2), slots):
            u()

        # qc3: qc2 tails + batch-1 x quarter 2, v4-v9, k2
        slots = {}
        add_tail_slots(slots, 0, 2)
        extra = []
        if b > 1:
            extra += [x_qtr_unit(1, 2, nc.sync)]
            extra += v_units(1, 4) + v_units(1, 5)
            extra += qk_units(1, "k", 2)
            for sbi in range(6, 10):
                extra += v_units(1, sbi)
        spread(slots, extra, 2, 15)
        for u in weave(attn_qc_units(0, 3), slots):
            u()

        if b > 1:
            # batch-1 qc0: norms(0,3) + x quarter 3, v10-v15, k3, q1 in
            # need-order (v_kb before step kb+1, k3 before step 12); the
            # ffs(0,3) are deferred into b1-qc1 slots 2-9
            slots = {}
            slots.setdefault(0, []).append(norm_unit(0, 3, 0))
            slots.setdefault(1, []).append(norm_unit(0, 3, 1))
            slots.setdefault(0, []).append(x_qtr_unit(1, 3, nc.sync))
            vdue = {2: 10, 3: 11, 6: 12, 8: 13, 10: 14, 11: 15}
            for pos, sbi in vdue.items():
                slots.setdefault(pos, []).extend(v_units(1, sbi))
            k3b = qk_units(1, "k", 3)
            for i in range(3):
                slots.setdefault(4 + i, []).extend(k3b[3 * i : 3 * (i + 1)])
            q1b = qk_units(1, "q", 1)
            slots.setdefault(12, []).extend(q1b[0:5])
            slots.setdefault(13, []).extend(q1b[5:])
            for j in range(KC):
                slots.setdefault(11 + j, []).append(ff_unit(0, 3, j))
            for u in weave(attn_qc_units(1, 0), slots):
                u()

            slots = {}
            add_tail_slots(slots, 1, 0)
            spread(slots, qk_units(1, "q", 2), 2, 8)
            for u in weave(attn_qc_units(1, 1), slots):
                u()

            slots = {}
            add_tail_slots(slots, 1, 1)
            spread(slots, qk_units(1, "q", 3), 2, 5)
            for u in weave(attn_qc_units(1, 2), slots):
                u()

            slots = {}
            add_tail_slots(slots, 1, 2)
            steps_last = attn_qc_units(1, 3)[:-1] + [attn_fin(1, 3, par=True)]
            for u in weave(steps_last, slots):
                u()

            # final tail: norm chains spread over four engines (o1 copy on
            # scalar in fin, dnrows on sync, muls on vector/gpsimd), PE warm
            # units spanning the chain, ffs with casts alternating
            # vector/scalar so the serial cast chain halves
            last = [
                norm_unit(1, 3, 0, dn_eng=nc.sync),
                norm_unit(1, 3, 1, dn_eng=nc.sync),
            ]
            last += [warm_unit(4) for _ in range(7)]
            last += [
                ff_unit(1, 3, j, cast_eng=(None if j % 2 == 0 else nc.scalar))
                for j in range(KC)
            ]
            for u in last:
                u()
        else:
            for u in tail_units(0, 3):
                u()

    nc.compile()
    return nc


def make_in_maps(x, Wq, Wk, Wv, Wff, n_cores=NCORES):
    """Per-core input dicts. Core c owns heads (2c, 2c+1) = D dims [128c, 128c+128)."""
    x = np.asarray(x, dtype=np.float32)
    b, s, d = x.shape
    KC = d // 128
    xT = np.ascontiguousarray(x.transpose(0, 2, 1)).reshape(b, KC, 128, s).astype(BF16)
    scale = 1.0 / np.sqrt(d)
    in_maps = []
    def pkm(wT):
        # (D, 128) transposed weight -> (128 partitions, KC, 128) contiguous
        return np.ascontiguousarray(wT.reshape(KC, 128, 128).transpose(1, 0, 2))

    for c in range(n_cores):
        sl = slice(128 * c, 128 * (c + 1))
        wq = pkm(np.ascontiguousarray((np.asarray(Wq)[sl, :] * scale).T))
        wk = pkm(np.ascontiguousarray(np.asarray(Wk)[sl, :].T))
        wv = pkm(np.ascontiguousarray(np.asarray(Wv)[sl, :].T))
        wf = np.ascontiguousarray(np.asarray(Wff)[:, sl].T).reshape(128, KC, 128)
        in_maps.append(
            {
                "xT": xT,
                "wqT": wq.astype(BF16),
                "wkT": wk.astype(BF16),
                "wvT": wv.astype(BF16),
                "wfT": wf.astype(BF16),
            }
        )
    return in_maps


def gather(results, bff, b=B, s=S, d=D):
    total = np.zeros((b, d // 128, 128, s), np.float32)
    for r in results:
        total += r["outp"].astype(np.float32)
    out = total.reshape(b, d, s).transpose(0, 2, 1)
    return (out + np.asarray(bff, np.float32)[None, None, :]).astype(np.float32)


_CACHE = {}


def kernel(x, Wq, Wk, Wv, Wff, bff):
    from concourse.bass_utils import run_bass_kernel_spmd

    x = np.asarray(x, np.float32)
    b, s, d = x.shape
    key = (b, s, d)
    if key not in _CACHE:
        _CACHE[key] = build_program(b, s, d)
    nc = _CACHE[key]
    in_maps = make_in_maps(x, Wq, Wk, Wv, Wff)
    res = run_bass_kernel_spmd(nc, in_maps, list(range(NCORES)))
    return gather(res.results, bff, b, s, d)
